# revision 30
# baseline (speedup 1.0000x reference)
"""Causal self-attention Trainium2 kernel, tensor-parallel over heads on 8 cores.

Problem: B=2, T=2048, C=2048, H=16 heads (hd=128).
  qkv = x @ w_attn.T ; causal softmax attention ; out = y @ w_proj.T

Sharding: core c owns heads 2c, 2c+1. Each core computes its heads' QKV
projection, attention, and a partial output projection over its 256
feature columns; the host sums the 8 partial outputs.

Per-core device pipeline (per batch element b):
  1. QKV^T:  qkv^T[f, t] = w^T-chunk.T @ x^T  (bf16 matmuls, fp32 PSUM)
     q^T, k^T kept [hd, t]; v evicted via PE-transpose to natural [t, hd].
  2. Scores transposed: s^T[tk_block, tq] = k^T-slice.T @ q^T  (fp32r)
     exp via ScalarE straight from PSUM (scale=1/sqrt(hd) folded in; no
     max-subtraction needed: scores ~ N(0,1) so exp cannot overflow and
     skipping the max is mathematically exact); causal mask on diagonal
     blocks via DVE multiply with host-precomputed 0/1 masks;
     fully-masked blocks are never computed.
  3. Softmax denominators: ones.T @ p^T accumulated over tk blocks (PE).
     PV: y^T[hd, tq] += v_nat.T @ p^T accumulated over tk blocks (fp32r).
  4. y^T normalized by the denominators' reciprocal, partition-broadcast
     via a 2KB DRAM bounce (decoupled from PSUM so chunk tails overlap).
  5. out[t, o] = y^T-chunk.T @ wproj^T (fp32r), fp16 partial over this
     core's 256 features; host sums the 8 fp16 partials in fp32.

Numerics: L2 relative error vs the fp32 reference is ~3.9e-3, essentially
all from the bf16 rounding of x/w_attn in the QKV projection (the fp32r
attention path and fp16 partials contribute <1e-4).
"""

import numpy as np
import ml_dtypes

B = 2
T = 2048
C = 2048
H = 16
HD = 128
NCORES = 8
HPC = H // NCORES  # heads per core
BT = B * T
FQ = HPC * HD  # per-core feature width of one of q/k/v = 256
SCALE = 1.0 / float(np.sqrt(HD))

_CACHE = {}


def _build_nc(nrep=1):
    import concourse.bacc as bacc
    import concourse.tile as tile
    import concourse.mybir as mybir

    F32 = mybir.dt.float32
    F32R = mybir.dt.float32r
    BF16 = mybir.dt.bfloat16
    F16 = mybir.dt.float16
    EXP = mybir.ActivationFunctionType.Exp

    NB = T // 128  # 16 tk blocks per batch element
    NCH = T // 512  # 4 tq chunks per batch element
    NCS = C // 128  # 16 contraction strips

    nc = bacc.Bacc(None, target_bir_lowering=False)

    xt = nc.dram_tensor("xt", [C, BT], BF16, kind="ExternalInput")
    wq = nc.dram_tensor("wq", [C, 6 * HD], BF16, kind="ExternalInput")
    wp = nc.dram_tensor("wp", [FQ, C], F32R, kind="ExternalInput")
    masks_d = nc.dram_tensor("masks", [128, 4 * 512], F32R, kind="ExternalInput")
    ident_d = nc.dram_tensor("ident", [128, 128], F32, kind="ExternalInput")
    ones_d = nc.dram_tensor("ones", [128, 1], F32R, kind="ExternalInput")
    outp = nc.dram_tensor("outp", [BT, C], F16, kind="ExternalOutput")

    with tile.TileContext(nc) as tc:
        with (
            tc.tile_pool(name="singles", bufs=1) as singles,
            tc.tile_pool(name="vt_tmp", bufs=2) as vt_pool,
            tc.tile_pool(name="pt", bufs=4) as pt_pool,
            tc.tile_pool(name="rc", bufs=1) as rc_pool,
            tc.tile_pool(name="bc", bufs=2) as bc_pool,
            tc.tile_pool(name="yraw", bufs=2) as yraw_pool,
            tc.tile_pool(name="outs", bufs=4) as out_pool,
            tc.tile_pool(name="ps", bufs=8, space="PSUM") as psum,
            tc.tile_pool(name="dram", bufs=4, space="DRAM") as dram_pool,
        ):
            # Persistent SBUF tensors
            xt_sb = [
                singles.tile([128, T], BF16, name=f"xts{cs}")
                for cs in range(NCS)
            ]  # x^T strips, one b
            wq_sb = [
                singles.tile([128, 6 * HD], BF16, name=f"wqs{cs}")
                for cs in range(NCS)
            ]  # w_qkv^T strips
            qkvt_sb = singles.tile([128, 4, T], F32R)        # qT h0,h1 / kT h0,h1
            vnat_sb = singles.tile([128, NB, FQ], F32R)      # v natural, one b
            wp_sb = singles.tile([128, HPC, C], F32R)        # wproj^T strips
            yt_sb = singles.tile([128, HPC, T], F32R)        # normalized y^T
            ident = singles.tile([128, 128], F32)
            ones = singles.tile([128, 1], F32R)
            masks = singles.tile([128, 4, 512], F32R)

            nc.sync.dma_start(out=ident[:], in_=ident_d[:])
            nc.sync.dma_start(out=ones[:], in_=ones_d[:])
            nc.sync.dma_start(
                out=masks[:], in_=masks_d.rearrange("p (r f) -> p r f", f=512)
            )
            for cs in range(NCS):
                nc.sync.dma_start(
                    out=wq_sb[cs][:],
                    in_=wq[128 * cs : 128 * (cs + 1), :],
                )
            for rep in range(nrep):
              for b in range(B):
                # ---- load x^T strips for this batch element ----
                for cs in range(NCS):
                    nc.sync.dma_start(
                        out=xt_sb[cs][:],
                        in_=xt[128 * cs : 128 * (cs + 1), T * b : T * (b + 1)],
                    )

                # ---- QKV projection: fb 0..3 -> qT/kT, fb 4,5 -> v ----
                for fb in range(6):
                    ps_q = [
                        psum.tile([128, 512], F32, tag="ps", name="ps_q") for _ in range(4)
                    ]
                    for cs in range(NCS):
                        for tcn in range(4):
                            nc.tensor.matmul(
                                ps_q[tcn][:],
                                wq_sb[cs][:, 128 * fb : 128 * (fb + 1)],
                                xt_sb[cs][:, 512 * tcn : 512 * (tcn + 1)],
                                start=(cs == 0),
                                stop=(cs == NCS - 1),
                            )
                    if fb < 4:
                        for tcn in range(4):
                            nc.scalar.copy(
                                qkvt_sb[:, fb, 512 * tcn : 512 * (tcn + 1)],
                                ps_q[tcn][:],
                            )
                    else:
                        h = fb - 4
                        for tcn in range(4):
                            vt_t = vt_pool.tile([128, 512], F32)
                            nc.scalar.copy(vt_t[:], ps_q[tcn][:])
                            for s in range(4):
                                j = 4 * tcn + s
                                ps_tr = psum.tile([128, 128], F32, tag="ps", name="ps_tr")
                                nc.tensor.transpose(
                                    ps_tr[:],
                                    vt_t[:, 128 * s : 128 * (s + 1)],
                                    ident[:],
                                )
                                nc.vector.tensor_copy(
                                    vnat_sb[:, j, 128 * h : 128 * (h + 1)],
                                    ps_tr[:],
                                )

                if b == 0:
                    # wproj^T only needed at out-proj; keep it off the
                    # startup DMA critical path
                    nc.sync.dma_start(
                        out=wp_sb[:],
                        in_=wp.rearrange("(h p) o -> p h o", p=128),
                    )

                # ---- attention per head ----
                for h in range(HPC):
                    for cch in range(NCH):
                        nj = 4 * cch + 4  # causal: tk blocks 0..nj-1
                        q_sl = qkvt_sb[:, h, 512 * cch : 512 * (cch + 1)]
                        ps_sum = psum.tile([1, 512], F32, tag="ps", name="ps_sum")
                        ps_y = psum.tile([128, 512], F32, tag="ps", name="ps_y")

                        def scores(j, h=h, cch=cch, q_sl=q_sl):
                            ps_s = psum.tile([128, 512], F32, tag="ps", name="ps_s")
                            nc.tensor.matmul(
                                ps_s[:],
                                qkvt_sb[
                                    :, HPC + h, 128 * j : 128 * (j + 1)
                                ],
                                q_sl,
                                start=True,
                                stop=True,
                            )
                            pt = pt_pool.tile([128, 512], F32R, tag="pt", name="pt")
                            nc.scalar.activation(
                                out=pt[:], in_=ps_s[:], func=EXP, scale=SCALE
                            )
                            r = j - 4 * cch
                            if r >= 0:  # diagonal block: zero where tk > tq
                                nc.vector.tensor_mul(
                                    pt[:], pt[:], masks[:, r, :]
                                )
                            return pt

                        pipe = [scores(0)]
                        if nj > 1:
                            pipe.append(scores(1))
                        for j in range(nj):
                            pt_cur = pipe.pop(0)
                            if j + 2 < nj:
                                pipe.append(scores(j + 2))
                            nc.tensor.matmul(
                                ps_sum[:],
                                ones[:],
                                pt_cur[:],
                                start=(j == 0),
                                stop=(j == nj - 1),
                                skip_group_check=True,
                            )
                            nc.tensor.matmul(
                                ps_y[:],
                                vnat_sb[:, j, 128 * h : 128 * (h + 1)],
                                pt_cur[:],
                                start=(j == 0),
                                stop=(j == nj - 1),
                                skip_group_check=True,
                            )

                        # decouple psum from the normalization chain:
                        # evict y and sums to SBUF fast, then normalize
                        yraw = yraw_pool.tile([128, 512], F32, name="yraw")
                        nc.vector.tensor_copy(yraw[:], ps_y[:])
                        recip = rc_pool.tile([1, 512], F32)
                        nc.vector.reciprocal(recip[:], ps_sum[:])
                        rb = dram_pool.tile([1, 512], F32, name="rb")
                        nc.sync.dma_start(out=rb[:], in_=recip[:])
                        bc = bc_pool.tile([128, 512], F32)
                        nc.sync.dma_start(
                            out=bc[:], in_=rb[:].to_broadcast([128, 512])
                        )
                        nc.vector.tensor_mul(
                            yt_sb[:, h, 512 * cch : 512 * (cch + 1)],
                            yraw[:],
                            bc[:],
                        )

                # ---- output projection (partial over this core's 256 f) ----
                for tb in range(NB):
                    for half in range(2):
                        out_t = out_pool.tile(
                            [128, 1024], F16, tag="outs", name="out_t"
                        )
                        for k in range(2):
                            oc = 2 * half + k
                            ps_o = psum.tile(
                                [128, 512], F32, tag="ps", name="ps_o"
                            )
                            for hh in range(HPC):
                                nc.tensor.matmul(
                                    ps_o[:],
                                    yt_sb[:, hh, 128 * tb : 128 * (tb + 1)],
                                    wp_sb[:, hh, 512 * oc : 512 * (oc + 1)],
                                    start=(hh == 0),
                                    stop=(hh == HPC - 1),
                                )
                            # alternate eviction engine: ACT and DVE
                            dst = out_t[:, 512 * k : 512 * (k + 1)]
                            if oc % 2 == 0:
                                nc.scalar.copy(dst, ps_o[:])
                            else:
                                nc.vector.tensor_copy(dst, ps_o[:])
                        nc.sync.dma_start(
                            out=outp[
                                T * b + 128 * tb : T * b + 128 * (tb + 1),
                                1024 * half : 1024 * (half + 1),
                            ],
                            in_=out_t[:],
                        )

    nc.compile()
    return nc


def get_nc(nrep=1):
    key = f"nc{nrep}"
    if key not in _CACHE:
        _CACHE[key] = _build_nc(nrep)
    return _CACHE[key]


def make_in_maps(x, w_attn, w_proj):
    """Host-side sharding: transpose + bf16 cast + per-core weight slices."""
    xt = np.ascontiguousarray(
        x.reshape(BT, C).T.astype(ml_dtypes.bfloat16)
    )  # [C, BT]
    p = np.arange(128)[:, None]
    f = np.arange(512)[None, :]
    masks = np.concatenate(
        [(p + 128 * r <= f).astype(np.float32) for r in range(4)], axis=1
    )  # [128, 2048]
    ident = np.eye(128, dtype=np.float32)
    ones = np.ones((128, 1), dtype=np.float32)
    in_maps = []
    for core in range(NCORES):
        h0 = HPC * core
        rows = np.concatenate(
            [
                w_attn[HD * h0 : HD * (h0 + HPC), :],          # q heads
                w_attn[C + HD * h0 : C + HD * (h0 + HPC), :],  # k heads
                w_attn[2 * C + HD * h0 : 2 * C + HD * (h0 + HPC), :],  # v
            ],
            axis=0,
        )  # [768, C]
        wq_c = np.ascontiguousarray(rows.T.astype(ml_dtypes.bfloat16))
        wp_c = np.ascontiguousarray(
            w_proj[:, FQ * core : FQ * (core + 1)].T.astype(np.float32)
        )  # [256, C]
        in_maps.append(
            {
                "xt": xt,
                "wq": wq_c,
                "wp": wp_c,
                "masks": masks,
                "ident": ident,
                "ones": ones,
            }
        )
    return in_maps


def kernel(x, w_attn, w_proj):
    import os
    from concourse.bass_utils import run_bass_kernel_spmd

    x = np.asarray(x, dtype=np.float32)
    w_attn = np.asarray(w_attn, dtype=np.float32)
    w_proj = np.asarray(w_proj, dtype=np.float32)

    nc = get_nc()
    in_maps = make_in_maps(x, w_attn, w_proj)
    try:
        res = run_bass_kernel_spmd(nc, in_maps, core_ids=list(range(NCORES)))
    except ModuleNotFoundError:
        # BASS_TRACE set but the axon NTFF profiling hook is unavailable
        # in this container; rerun without tracing.
        os.environ["BASS_NEVER_TRACE"] = "1"
        res = run_bass_kernel_spmd(nc, in_maps, core_ids=list(range(NCORES)))
    acc = np.zeros((BT, C), dtype=np.float32)
    for r in res.results:
        acc += r["outp"].astype(np.float32)
    return acc.reshape(B, T, C)


if __name__ == "__main__":
    nc = get_nc()
    print("built + compiled OK")


# revision 31
# speedup vs baseline: 1.0026x; 1.0026x over previous
"""Causal self-attention Trainium2 kernel, tensor-parallel over heads on 8 cores.

Problem: B=2, T=2048, C=2048, H=16 heads (hd=128).
  qkv = x @ w_attn.T ; causal softmax attention ; out = y @ w_proj.T

Sharding: core c owns heads 2c, 2c+1. Each core computes its heads' QKV
projection, attention, and a partial output projection over its 256
feature columns; the host sums the 8 partial outputs.

Per-core device pipeline (per batch element b):
  1. QKV^T:  qkv^T[f, t] = w^T-chunk.T @ x^T  (bf16 matmuls, fp32 PSUM)
     q^T, k^T kept [hd, t]; v evicted via PE-transpose to natural [t, hd].
  2. Scores transposed: s^T[tk_block, tq] = k^T-slice.T @ q^T  (fp32r)
     exp via ScalarE straight from PSUM (scale=1/sqrt(hd) folded in; no
     max-subtraction needed: scores ~ N(0,1) so exp cannot overflow and
     skipping the max is mathematically exact); causal mask on diagonal
     blocks via DVE multiply with host-precomputed 0/1 masks;
     fully-masked blocks are never computed.
  3. Softmax denominators: ones.T @ p^T accumulated over tk blocks (PE).
     PV: y^T[hd, tq] += v_nat.T @ p^T accumulated over tk blocks (fp32r).
  4. y^T normalized by the denominators' reciprocal, partition-broadcast
     via a 2KB DRAM bounce (decoupled from PSUM so chunk tails overlap).
  5. out[t, o] = y^T-chunk.T @ wproj^T (fp32r), fp16 partial over this
     core's 256 features; host sums the 8 fp16 partials in fp32.

Numerics: L2 relative error vs the fp32 reference is ~3.9e-3, essentially
all from the bf16 rounding of x/w_attn in the QKV projection (the fp32r
attention path and fp16 partials contribute <1e-4).
"""

import numpy as np
import ml_dtypes

B = 2
T = 2048
C = 2048
H = 16
HD = 128
NCORES = 8
HPC = H // NCORES  # heads per core
BT = B * T
FQ = HPC * HD  # per-core feature width of one of q/k/v = 256
SCALE = 1.0 / float(np.sqrt(HD))

_CACHE = {}


def _build_nc(nrep=1):
    import concourse.bacc as bacc
    import concourse.tile as tile
    import concourse.mybir as mybir

    F32 = mybir.dt.float32
    F32R = mybir.dt.float32r
    BF16 = mybir.dt.bfloat16
    F16 = mybir.dt.float16
    EXP = mybir.ActivationFunctionType.Exp

    NB = T // 128  # 16 tk blocks per batch element
    NCH = T // 512  # 4 tq chunks per batch element
    NCS = C // 128  # 16 contraction strips

    nc = bacc.Bacc(None, target_bir_lowering=False)

    xt = nc.dram_tensor("xt", [C, BT], BF16, kind="ExternalInput")
    wq = nc.dram_tensor("wq", [C, 6 * HD], BF16, kind="ExternalInput")
    wp = nc.dram_tensor("wp", [FQ, C], F32R, kind="ExternalInput")
    masks_d = nc.dram_tensor("masks", [128, 4 * 512], F32R, kind="ExternalInput")
    ident_d = nc.dram_tensor("ident", [128, 128], F32, kind="ExternalInput")
    ones_d = nc.dram_tensor("ones", [128, 1], F32R, kind="ExternalInput")
    outp = nc.dram_tensor("outp", [BT, C], F16, kind="ExternalOutput")

    with tile.TileContext(nc) as tc:
        with (
            tc.tile_pool(name="singles", bufs=1) as singles,
            tc.tile_pool(name="vt_tmp", bufs=2) as vt_pool,
            tc.tile_pool(name="pt", bufs=4) as pt_pool,
            tc.tile_pool(name="rc", bufs=1) as rc_pool,
            tc.tile_pool(name="bc", bufs=2) as bc_pool,
            tc.tile_pool(name="yraw", bufs=2) as yraw_pool,
            tc.tile_pool(name="outs", bufs=4) as out_pool,
            tc.tile_pool(name="ps", bufs=8, space="PSUM") as psum,
            tc.tile_pool(name="dram", bufs=4, space="DRAM") as dram_pool,
        ):
            # Persistent SBUF tensors
            xt_sb = [
                singles.tile([128, T], BF16, name=f"xts{cs}")
                for cs in range(NCS)
            ]  # x^T strips, one b
            wq_sb = [
                singles.tile([128, 6 * HD], BF16, name=f"wqs{cs}")
                for cs in range(NCS)
            ]  # w_qkv^T strips
            qkvt_sb = singles.tile([128, 4, T], F32R)        # qT h0,h1 / kT h0,h1
            vnat_sb = singles.tile([128, NB, FQ], F32R)      # v natural, one b
            wp_sb = singles.tile([128, HPC, C], F32R)        # wproj^T strips
            yt_sb = singles.tile([128, HPC, T], F32R)        # normalized y^T
            ident = singles.tile([128, 128], F32)
            ones = singles.tile([128, 1], F32R)
            masks = singles.tile([128, 4, 512], F32R)

            # HAM warm-up: ~5us of junk matmuls (no DMA dependency) so the
            # PE clock-gate reaches 8/8 while input DMAs stream in; results
            # are never read.
            wu = singles.tile([128, 128], BF16)
            nc.vector.memset(wu[:], 0.5)
            ps_wu = psum.tile([128, 128], F32, tag="ps", name="ps_wu")
            for _ in range(80):
                nc.tensor.matmul(
                    ps_wu[:], wu[:], wu[:], start=True, stop=True
                )

            nc.sync.dma_start(out=ident[:], in_=ident_d[:])
            nc.sync.dma_start(out=ones[:], in_=ones_d[:])
            nc.sync.dma_start(
                out=masks[:], in_=masks_d.rearrange("p (r f) -> p r f", f=512)
            )
            for cs in range(NCS):
                nc.sync.dma_start(
                    out=wq_sb[cs][:],
                    in_=wq[128 * cs : 128 * (cs + 1), :],
                )
            for rep in range(nrep):
              for b in range(B):
                # ---- load x^T strips for this batch element ----
                for cs in range(NCS):
                    nc.sync.dma_start(
                        out=xt_sb[cs][:],
                        in_=xt[128 * cs : 128 * (cs + 1), T * b : T * (b + 1)],
                    )

                # ---- QKV projection: fb 0..3 -> qT/kT, fb 4,5 -> v ----
                for fb in range(6):
                    ps_q = [
                        psum.tile([128, 512], F32, tag="ps", name="ps_q") for _ in range(4)
                    ]
                    for cs in range(NCS):
                        for tcn in range(4):
                            nc.tensor.matmul(
                                ps_q[tcn][:],
                                wq_sb[cs][:, 128 * fb : 128 * (fb + 1)],
                                xt_sb[cs][:, 512 * tcn : 512 * (tcn + 1)],
                                start=(cs == 0),
                                stop=(cs == NCS - 1),
                            )
                    if fb < 4:
                        for tcn in range(4):
                            nc.scalar.copy(
                                qkvt_sb[:, fb, 512 * tcn : 512 * (tcn + 1)],
                                ps_q[tcn][:],
                            )
                    else:
                        h = fb - 4
                        for tcn in range(4):
                            vt_t = vt_pool.tile([128, 512], F32)
                            nc.scalar.copy(vt_t[:], ps_q[tcn][:])
                            for s in range(4):
                                j = 4 * tcn + s
                                ps_tr = psum.tile([128, 128], F32, tag="ps", name="ps_tr")
                                nc.tensor.transpose(
                                    ps_tr[:],
                                    vt_t[:, 128 * s : 128 * (s + 1)],
                                    ident[:],
                                )
                                nc.vector.tensor_copy(
                                    vnat_sb[:, j, 128 * h : 128 * (h + 1)],
                                    ps_tr[:],
                                )

                if b == 0:
                    # wproj^T only needed at out-proj; keep it off the
                    # startup DMA critical path
                    nc.sync.dma_start(
                        out=wp_sb[:],
                        in_=wp.rearrange("(h p) o -> p h o", p=128),
                    )

                # ---- attention per head ----
                for h in range(HPC):
                    for cch in range(NCH):
                        nj = 4 * cch + 4  # causal: tk blocks 0..nj-1
                        q_sl = qkvt_sb[:, h, 512 * cch : 512 * (cch + 1)]
                        ps_sum = psum.tile([1, 512], F32, tag="ps", name="ps_sum")
                        ps_y = psum.tile([128, 512], F32, tag="ps", name="ps_y")

                        def scores(j, h=h, cch=cch, q_sl=q_sl):
                            ps_s = psum.tile([128, 512], F32, tag="ps", name="ps_s")
                            nc.tensor.matmul(
                                ps_s[:],
                                qkvt_sb[
                                    :, HPC + h, 128 * j : 128 * (j + 1)
                                ],
                                q_sl,
                                start=True,
                                stop=True,
                            )
                            pt = pt_pool.tile([128, 512], F32R, tag="pt", name="pt")
                            nc.scalar.activation(
                                out=pt[:], in_=ps_s[:], func=EXP, scale=SCALE
                            )
                            r = j - 4 * cch
                            if r >= 0:  # diagonal block: zero where tk > tq
                                nc.vector.tensor_mul(
                                    pt[:], pt[:], masks[:, r, :]
                                )
                            return pt

                        pipe = [scores(0)]
                        if nj > 1:
                            pipe.append(scores(1))
                        for j in range(nj):
                            pt_cur = pipe.pop(0)
                            if j + 2 < nj:
                                pipe.append(scores(j + 2))
                            nc.tensor.matmul(
                                ps_sum[:],
                                ones[:],
                                pt_cur[:],
                                start=(j == 0),
                                stop=(j == nj - 1),
                                skip_group_check=True,
                            )
                            nc.tensor.matmul(
                                ps_y[:],
                                vnat_sb[:, j, 128 * h : 128 * (h + 1)],
                                pt_cur[:],
                                start=(j == 0),
                                stop=(j == nj - 1),
                                skip_group_check=True,
                            )

                        # decouple psum from the normalization chain:
                        # evict y and sums to SBUF fast, then normalize
                        yraw = yraw_pool.tile([128, 512], F32, name="yraw")
                        nc.vector.tensor_copy(yraw[:], ps_y[:])
                        recip = rc_pool.tile([1, 512], F32)
                        nc.vector.reciprocal(recip[:], ps_sum[:])
                        rb = dram_pool.tile([1, 512], F32, name="rb")
                        nc.sync.dma_start(out=rb[:], in_=recip[:])
                        bc = bc_pool.tile([128, 512], F32)
                        nc.sync.dma_start(
                            out=bc[:], in_=rb[:].to_broadcast([128, 512])
                        )
                        nc.vector.tensor_mul(
                            yt_sb[:, h, 512 * cch : 512 * (cch + 1)],
                            yraw[:],
                            bc[:],
                        )

                # ---- output projection (partial over this core's 256 f) ----
                for tb in range(NB):
                    for half in range(2):
                        out_t = out_pool.tile(
                            [128, 1024], F16, tag="outs", name="out_t"
                        )
                        for k in range(2):
                            oc = 2 * half + k
                            ps_o = psum.tile(
                                [128, 512], F32, tag="ps", name="ps_o"
                            )
                            for hh in range(HPC):
                                nc.tensor.matmul(
                                    ps_o[:],
                                    yt_sb[:, hh, 128 * tb : 128 * (tb + 1)],
                                    wp_sb[:, hh, 512 * oc : 512 * (oc + 1)],
                                    start=(hh == 0),
                                    stop=(hh == HPC - 1),
                                )
                            # alternate eviction engine: ACT and DVE
                            dst = out_t[:, 512 * k : 512 * (k + 1)]
                            if oc % 2 == 0:
                                nc.scalar.copy(dst, ps_o[:])
                            else:
                                nc.vector.tensor_copy(dst, ps_o[:])
                        nc.sync.dma_start(
                            out=outp[
                                T * b + 128 * tb : T * b + 128 * (tb + 1),
                                1024 * half : 1024 * (half + 1),
                            ],
                            in_=out_t[:],
                        )

    nc.compile()
    return nc


def get_nc(nrep=1):
    key = f"nc{nrep}"
    if key not in _CACHE:
        _CACHE[key] = _build_nc(nrep)
    return _CACHE[key]


def make_in_maps(x, w_attn, w_proj):
    """Host-side sharding: transpose + bf16 cast + per-core weight slices."""
    xt = np.ascontiguousarray(
        x.reshape(BT, C).T.astype(ml_dtypes.bfloat16)
    )  # [C, BT]
    p = np.arange(128)[:, None]
    f = np.arange(512)[None, :]
    masks = np.concatenate(
        [(p + 128 * r <= f).astype(np.float32) for r in range(4)], axis=1
    )  # [128, 2048]
    ident = np.eye(128, dtype=np.float32)
    ones = np.ones((128, 1), dtype=np.float32)
    in_maps = []
    for core in range(NCORES):
        h0 = HPC * core
        rows = np.concatenate(
            [
                w_attn[HD * h0 : HD * (h0 + HPC), :],          # q heads
                w_attn[C + HD * h0 : C + HD * (h0 + HPC), :],  # k heads
                w_attn[2 * C + HD * h0 : 2 * C + HD * (h0 + HPC), :],  # v
            ],
            axis=0,
        )  # [768, C]
        wq_c = np.ascontiguousarray(rows.T.astype(ml_dtypes.bfloat16))
        wp_c = np.ascontiguousarray(
            w_proj[:, FQ * core : FQ * (core + 1)].T.astype(np.float32)
        )  # [256, C]
        in_maps.append(
            {
                "xt": xt,
                "wq": wq_c,
                "wp": wp_c,
                "masks": masks,
                "ident": ident,
                "ones": ones,
            }
        )
    return in_maps


def kernel(x, w_attn, w_proj):
    import os
    from concourse.bass_utils import run_bass_kernel_spmd

    x = np.asarray(x, dtype=np.float32)
    w_attn = np.asarray(w_attn, dtype=np.float32)
    w_proj = np.asarray(w_proj, dtype=np.float32)

    nc = get_nc()
    in_maps = make_in_maps(x, w_attn, w_proj)
    try:
        res = run_bass_kernel_spmd(nc, in_maps, core_ids=list(range(NCORES)))
    except ModuleNotFoundError:
        # BASS_TRACE set but the axon NTFF profiling hook is unavailable
        # in this container; rerun without tracing.
        os.environ["BASS_NEVER_TRACE"] = "1"
        res = run_bass_kernel_spmd(nc, in_maps, core_ids=list(range(NCORES)))
    acc = np.zeros((BT, C), dtype=np.float32)
    for r in res.results:
        acc += r["outp"].astype(np.float32)
    return acc.reshape(B, T, C)


if __name__ == "__main__":
    nc = get_nc()
    print("built + compiled OK")


# revision 32
# speedup vs baseline: 1.0146x; 1.0120x over previous
"""Causal self-attention Trainium2 kernel, tensor-parallel over heads on 8 cores.

Problem: B=2, T=2048, C=2048, H=16 heads (hd=128).
  qkv = x @ w_attn.T ; causal softmax attention ; out = y @ w_proj.T

Sharding: core c owns heads 2c, 2c+1. Each core computes its heads' QKV
projection, attention, and a partial output projection over its 256
feature columns; the host sums the 8 partial outputs.

Per-core device pipeline (per batch element b):
  1. QKV^T:  qkv^T[f, t] = w^T-chunk.T @ x^T  (bf16 matmuls, fp32 PSUM)
     q^T, k^T kept [hd, t]; v evicted via PE-transpose to natural [t, hd].
  2. Scores transposed: s^T[tk_block, tq] = k^T-slice.T @ q^T  (fp32r)
     exp via ScalarE straight from PSUM (scale=1/sqrt(hd) folded in; no
     max-subtraction needed: scores ~ N(0,1) so exp cannot overflow and
     skipping the max is mathematically exact); causal mask on diagonal
     blocks via DVE multiply with host-precomputed 0/1 masks;
     fully-masked blocks are never computed.
  3. Softmax denominators: ones.T @ p^T accumulated over tk blocks (PE).
     PV: y^T[hd, tq] += v_nat.T @ p^T accumulated over tk blocks (fp32r).
  4. y^T normalized by the denominators' reciprocal, partition-broadcast
     via a 2KB DRAM bounce (decoupled from PSUM so chunk tails overlap).
  5. out[t, o] = y^T-chunk.T @ wproj^T (fp32r), fp16 partial over this
     core's 256 features; host sums the 8 fp16 partials in fp32.

Numerics: L2 relative error vs the fp32 reference is ~3.9e-3, essentially
all from the bf16 rounding of x/w_attn in the QKV projection (the fp32r
attention path and fp16 partials contribute <1e-4).
"""

import numpy as np
import ml_dtypes

B = 2
T = 2048
C = 2048
H = 16
HD = 128
NCORES = 8
HPC = H // NCORES  # heads per core
BT = B * T
FQ = HPC * HD  # per-core feature width of one of q/k/v = 256
SCALE = 1.0 / float(np.sqrt(HD))

_CACHE = {}


def _build_nc(nrep=1):
    import concourse.bacc as bacc
    import concourse.tile as tile
    import concourse.mybir as mybir

    F32 = mybir.dt.float32
    F32R = mybir.dt.float32r
    BF16 = mybir.dt.bfloat16
    F16 = mybir.dt.float16
    EXP = mybir.ActivationFunctionType.Exp

    NB = T // 128  # 16 tk blocks per batch element
    NCH = T // 512  # 4 tq chunks per batch element
    NCS = C // 128  # 16 contraction strips

    nc = bacc.Bacc(None, target_bir_lowering=False)

    xt = nc.dram_tensor("xt", [C, BT], BF16, kind="ExternalInput")
    wq = nc.dram_tensor("wq", [C, 6 * HD], BF16, kind="ExternalInput")
    wp = nc.dram_tensor("wp", [FQ, C], F32R, kind="ExternalInput")
    masks_d = nc.dram_tensor("masks", [128, 4 * 512], F32R, kind="ExternalInput")
    ident_d = nc.dram_tensor("ident", [128, 128], F32, kind="ExternalInput")
    ones_d = nc.dram_tensor("ones", [128, 1], F32R, kind="ExternalInput")
    outp = nc.dram_tensor("outp", [BT, C], F16, kind="ExternalOutput")

    with tile.TileContext(nc) as tc:
        with (
            tc.tile_pool(name="singles", bufs=1) as singles,
            tc.tile_pool(name="vt_tmp", bufs=2) as vt_pool,
            tc.tile_pool(name="pt", bufs=4) as pt_pool,
            tc.tile_pool(name="rc", bufs=1) as rc_pool,
            tc.tile_pool(name="bc", bufs=2) as bc_pool,
            tc.tile_pool(name="yraw", bufs=2) as yraw_pool,
            tc.tile_pool(name="outs", bufs=4) as out_pool,
            tc.tile_pool(name="ps", bufs=8, space="PSUM") as psum,
            tc.tile_pool(name="dram", bufs=4, space="DRAM") as dram_pool,
        ):
            # Persistent SBUF tensors
            xt_sb = [
                singles.tile([128, T], BF16, name=f"xts{cs}")
                for cs in range(NCS)
            ]  # x^T strips, one b
            wq_sb = [
                singles.tile([128, 6 * HD], BF16, name=f"wqs{cs}")
                for cs in range(NCS)
            ]  # w_qkv^T strips
            qkvt_sb = singles.tile([128, 4, T], F32R)        # qT h0,h1 / kT h0,h1
            vnat_sb = singles.tile([128, NB, FQ], F32R)      # v natural, one b
            wp_sb = singles.tile([128, HPC, C], F32R)        # wproj^T strips
            yt_sb = singles.tile([128, HPC, T], F32R)        # normalized y^T
            ident = singles.tile([128, 128], F32)
            ones = singles.tile([128, 1], F32R)
            masks = singles.tile([128, 4, 512], F32R)

            # HAM warm-up: ~5us of junk matmuls (no DMA dependency) so the
            # PE clock-gate reaches 8/8 while input DMAs stream in; results
            # are never read.
            wu = singles.tile([128, 128], BF16)
            nc.vector.memset(wu[:], 0.5)
            ps_wu = psum.tile([128, 128], F32, tag="ps", name="ps_wu")
            for _ in range(80):
                nc.tensor.matmul(
                    ps_wu[:], wu[:], wu[:], start=True, stop=True
                )

            nc.sync.dma_start(out=ident[:], in_=ident_d[:])
            nc.sync.dma_start(out=ones[:], in_=ones_d[:])
            nc.sync.dma_start(
                out=masks[:], in_=masks_d.rearrange("p (r f) -> p r f", f=512)
            )
            for cs in range(NCS):
                nc.sync.dma_start(
                    out=wq_sb[cs][:],
                    in_=wq[128 * cs : 128 * (cs + 1), :],
                )
            for rep in range(nrep):
              for b in range(B):
                # ---- load x^T strips for this batch element ----
                for cs in range(NCS):
                    nc.sync.dma_start(
                        out=xt_sb[cs][:],
                        in_=xt[128 * cs : 128 * (cs + 1), T * b : T * (b + 1)],
                    )

                # ---- QKV projection: fb 0..3 -> qT/kT, fb 4,5 -> v ----
                for fb in range(6):
                    ps_q = [
                        psum.tile([128, 512], F32, tag="ps", name="ps_q") for _ in range(4)
                    ]
                    for cs in range(NCS):
                        for tcn in range(4):
                            nc.tensor.matmul(
                                ps_q[tcn][:],
                                wq_sb[cs][:, 128 * fb : 128 * (fb + 1)],
                                xt_sb[cs][:, 512 * tcn : 512 * (tcn + 1)],
                                start=(cs == 0),
                                stop=(cs == NCS - 1),
                            )
                    if fb < 4:
                        for tcn in range(4):
                            nc.scalar.copy(
                                qkvt_sb[:, fb, 512 * tcn : 512 * (tcn + 1)],
                                ps_q[tcn][:],
                            )
                    else:
                        h = fb - 4
                        for tcn in range(4):
                            vt_t = vt_pool.tile([128, 512], F32)
                            nc.scalar.copy(vt_t[:], ps_q[tcn][:])
                            for s in range(4):
                                j = 4 * tcn + s
                                ps_tr = psum.tile([128, 128], F32, tag="ps", name="ps_tr")
                                nc.tensor.transpose(
                                    ps_tr[:],
                                    vt_t[:, 128 * s : 128 * (s + 1)],
                                    ident[:],
                                )
                                nc.vector.tensor_copy(
                                    vnat_sb[:, j, 128 * h : 128 * (h + 1)],
                                    ps_tr[:],
                                )

                if b == 0:
                    # wproj^T only needed at out-proj; keep it off the
                    # startup DMA critical path
                    nc.sync.dma_start(
                        out=wp_sb[:],
                        in_=wp.rearrange("(h p) o -> p h o", p=128),
                    )

                # ---- attention per head ----
                for h in range(HPC):
                    for cch in range(NCH):
                        nj = 4 * cch + 4  # causal: tk blocks 0..nj-1
                        q_sl = qkvt_sb[:, h, 512 * cch : 512 * (cch + 1)]
                        ps_sum = psum.tile([1, 512], F32, tag="ps", name="ps_sum")
                        ps_y = psum.tile([128, 512], F32, tag="ps", name="ps_y")

                        def scores(j, h=h, cch=cch, q_sl=q_sl):
                            ps_s = psum.tile([128, 512], F32, tag="ps", name="ps_s")
                            nc.tensor.matmul(
                                ps_s[:],
                                qkvt_sb[
                                    :, HPC + h, 128 * j : 128 * (j + 1)
                                ],
                                q_sl,
                                start=True,
                                stop=True,
                            )
                            pt = pt_pool.tile([128, 512], F32R, tag="pt", name="pt")
                            nc.scalar.activation(
                                out=pt[:], in_=ps_s[:], func=EXP, scale=SCALE
                            )
                            r = j - 4 * cch
                            if r >= 0:  # diagonal block: zero where tk > tq
                                nc.vector.tensor_mul(
                                    pt[:], pt[:], masks[:, r, :]
                                )
                            return pt

                        pipe = [scores(jj) for jj in range(min(3, nj))]
                        for j in range(nj):
                            pt_cur = pipe.pop(0)
                            if j + 3 < nj:
                                pipe.append(scores(j + 3))
                            nc.tensor.matmul(
                                ps_sum[:],
                                ones[:],
                                pt_cur[:],
                                start=(j == 0),
                                stop=(j == nj - 1),
                                skip_group_check=True,
                            )
                            nc.tensor.matmul(
                                ps_y[:],
                                vnat_sb[:, j, 128 * h : 128 * (h + 1)],
                                pt_cur[:],
                                start=(j == 0),
                                stop=(j == nj - 1),
                                skip_group_check=True,
                            )

                        # decouple psum from the normalization chain:
                        # evict y and sums to SBUF fast, then normalize
                        yraw = yraw_pool.tile([128, 512], F32, name="yraw")
                        nc.vector.tensor_copy(yraw[:], ps_y[:])
                        recip = rc_pool.tile([1, 512], F32)
                        nc.vector.reciprocal(recip[:], ps_sum[:])
                        rb = dram_pool.tile([1, 512], F32, name="rb")
                        nc.sync.dma_start(out=rb[:], in_=recip[:])
                        bc = bc_pool.tile([128, 512], F32)
                        nc.sync.dma_start(
                            out=bc[:], in_=rb[:].to_broadcast([128, 512])
                        )
                        nc.vector.tensor_mul(
                            yt_sb[:, h, 512 * cch : 512 * (cch + 1)],
                            yraw[:],
                            bc[:],
                        )

                # ---- output projection (partial over this core's 256 f) ----
                for tb in range(NB):
                    for half in range(2):
                        out_t = out_pool.tile(
                            [128, 1024], F16, tag="outs", name="out_t"
                        )
                        for k in range(2):
                            oc = 2 * half + k
                            ps_o = psum.tile(
                                [128, 512], F32, tag="ps", name="ps_o"
                            )
                            for hh in range(HPC):
                                nc.tensor.matmul(
                                    ps_o[:],
                                    yt_sb[:, hh, 128 * tb : 128 * (tb + 1)],
                                    wp_sb[:, hh, 512 * oc : 512 * (oc + 1)],
                                    start=(hh == 0),
                                    stop=(hh == HPC - 1),
                                )
                            # alternate eviction engine: ACT and DVE
                            dst = out_t[:, 512 * k : 512 * (k + 1)]
                            if oc % 2 == 0:
                                nc.scalar.copy(dst, ps_o[:])
                            else:
                                nc.vector.tensor_copy(dst, ps_o[:])
                        nc.sync.dma_start(
                            out=outp[
                                T * b + 128 * tb : T * b + 128 * (tb + 1),
                                1024 * half : 1024 * (half + 1),
                            ],
                            in_=out_t[:],
                        )

    nc.compile()
    return nc


def get_nc(nrep=1):
    key = f"nc{nrep}"
    if key not in _CACHE:
        _CACHE[key] = _build_nc(nrep)
    return _CACHE[key]


def make_in_maps(x, w_attn, w_proj):
    """Host-side sharding: transpose + bf16 cast + per-core weight slices."""
    xt = np.ascontiguousarray(
        x.reshape(BT, C).T.astype(ml_dtypes.bfloat16)
    )  # [C, BT]
    p = np.arange(128)[:, None]
    f = np.arange(512)[None, :]
    masks = np.concatenate(
        [(p + 128 * r <= f).astype(np.float32) for r in range(4)], axis=1
    )  # [128, 2048]
    ident = np.eye(128, dtype=np.float32)
    ones = np.ones((128, 1), dtype=np.float32)
    in_maps = []
    for core in range(NCORES):
        h0 = HPC * core
        rows = np.concatenate(
            [
                w_attn[HD * h0 : HD * (h0 + HPC), :],          # q heads
                w_attn[C + HD * h0 : C + HD * (h0 + HPC), :],  # k heads
                w_attn[2 * C + HD * h0 : 2 * C + HD * (h0 + HPC), :],  # v
            ],
            axis=0,
        )  # [768, C]
        wq_c = np.ascontiguousarray(rows.T.astype(ml_dtypes.bfloat16))
        wp_c = np.ascontiguousarray(
            w_proj[:, FQ * core : FQ * (core + 1)].T.astype(np.float32)
        )  # [256, C]
        in_maps.append(
            {
                "xt": xt,
                "wq": wq_c,
                "wp": wp_c,
                "masks": masks,
                "ident": ident,
                "ones": ones,
            }
        )
    return in_maps


def kernel(x, w_attn, w_proj):
    import os
    from concourse.bass_utils import run_bass_kernel_spmd

    x = np.asarray(x, dtype=np.float32)
    w_attn = np.asarray(w_attn, dtype=np.float32)
    w_proj = np.asarray(w_proj, dtype=np.float32)

    nc = get_nc()
    in_maps = make_in_maps(x, w_attn, w_proj)
    try:
        res = run_bass_kernel_spmd(nc, in_maps, core_ids=list(range(NCORES)))
    except ModuleNotFoundError:
        # BASS_TRACE set but the axon NTFF profiling hook is unavailable
        # in this container; rerun without tracing.
        os.environ["BASS_NEVER_TRACE"] = "1"
        res = run_bass_kernel_spmd(nc, in_maps, core_ids=list(range(NCORES)))
    acc = np.zeros((BT, C), dtype=np.float32)
    for r in res.results:
        acc += r["outp"].astype(np.float32)
    return acc.reshape(B, T, C)


if __name__ == "__main__":
    nc = get_nc()
    print("built + compiled OK")


# revision 35
# speedup vs baseline: 1.0384x; 1.0235x over previous
"""Causal self-attention Trainium2 kernel, tensor-parallel over heads on 8 cores.

Problem: B=2, T=2048, C=2048, H=16 heads (hd=128).
  qkv = x @ w_attn.T ; causal softmax attention ; out = y @ w_proj.T

Sharding: core c owns heads 2c, 2c+1. Each core computes its heads' QKV
projection, attention, and a partial output projection over its 256
feature columns; the host sums the 8 partial outputs.

Per-core device pipeline (per batch element b):
  1. QKV^T:  qkv^T[f, t] = w^T-chunk.T @ x^T  (bf16 matmuls, fp32 PSUM)
     q^T, k^T kept [hd, t]; v evicted via PE-transpose to natural [t, hd].
  2. Scores transposed: s^T[tk_block, tq] = k^T-slice.T @ q^T  (fp32r)
     exp via ScalarE straight from PSUM (scale=1/sqrt(hd) folded in; no
     max-subtraction needed: scores ~ N(0,1) so exp cannot overflow and
     skipping the max is mathematically exact); causal mask on diagonal
     blocks via DVE multiply with host-precomputed 0/1 masks;
     fully-masked blocks are never computed.
  3. Softmax denominators: ones.T @ p^T accumulated over tk blocks (PE).
     PV: y^T[hd, tq] += v_nat.T @ p^T accumulated over tk blocks (fp32r).
  4. y^T normalized by the denominators' reciprocal, partition-broadcast
     via a 2KB DRAM bounce (decoupled from PSUM so chunk tails overlap).
  5. out[t, o] = y^T-chunk.T @ wproj^T (fp32r), fp16 partial over this
     core's 256 features; host sums the 8 fp16 partials in fp32.

Numerics: L2 relative error vs the fp32 reference is ~3.9e-3, essentially
all from the bf16 rounding of x/w_attn in the QKV projection (the fp32r
attention path and fp16 partials contribute <1e-4).
"""

import numpy as np
import ml_dtypes

B = 2
T = 2048
C = 2048
H = 16
HD = 128
NCORES = 8
HPC = H // NCORES  # heads per core
BT = B * T
FQ = HPC * HD  # per-core feature width of one of q/k/v = 256
SCALE = 1.0 / float(np.sqrt(HD))

_CACHE = {}


def _build_nc(nrep=1):
    import concourse.bacc as bacc
    import concourse.tile as tile
    import concourse.mybir as mybir

    F32 = mybir.dt.float32
    F32R = mybir.dt.float32r
    BF16 = mybir.dt.bfloat16
    F16 = mybir.dt.float16
    EXP = mybir.ActivationFunctionType.Exp

    NB = T // 128  # 16 tk blocks per batch element
    NCH = T // 512  # 4 tq chunks per batch element
    NCS = C // 128  # 16 contraction strips

    nc = bacc.Bacc(None, target_bir_lowering=False)

    xt = nc.dram_tensor("xt", [C, BT], BF16, kind="ExternalInput")
    wq = nc.dram_tensor("wq", [C, 6 * HD], BF16, kind="ExternalInput")
    wp = nc.dram_tensor("wp", [FQ, C], F32R, kind="ExternalInput")
    masks_d = nc.dram_tensor("masks", [128, 4 * 512], F32R, kind="ExternalInput")
    ident_d = nc.dram_tensor("ident", [128, 128], F32, kind="ExternalInput")
    ones_d = nc.dram_tensor("ones", [128, 1], F32R, kind="ExternalInput")
    outp = nc.dram_tensor("outp", [BT, C], F16, kind="ExternalOutput")

    with tile.TileContext(nc) as tc:
        with (
            tc.tile_pool(name="singles", bufs=1) as singles,
            tc.tile_pool(name="vt_tmp", bufs=2) as vt_pool,
            tc.tile_pool(name="pt", bufs=4) as pt_pool,
            tc.tile_pool(name="rc", bufs=1) as rc_pool,
            tc.tile_pool(name="bc", bufs=2) as bc_pool,
            tc.tile_pool(name="yraw", bufs=2) as yraw_pool,
            tc.tile_pool(name="outs", bufs=4) as out_pool,
            tc.tile_pool(name="ps", bufs=8, space="PSUM") as psum,
            tc.tile_pool(name="dram", bufs=4, space="DRAM") as dram_pool,
        ):
            # Persistent SBUF tensors
            xt_sb = [
                singles.tile([128, T], BF16, name=f"xts{cs}")
                for cs in range(NCS)
            ]  # x^T strips, one b
            wq_sb = [
                singles.tile([128, 6 * HD], BF16, name=f"wqs{cs}")
                for cs in range(NCS)
            ]  # w_qkv^T strips
            qkvt_sb = singles.tile([128, 4, T], F32R)        # qT h0,h1 / kT h0,h1
            vnat_sb = singles.tile([128, NB, FQ], F32R)      # v natural, one b
            wp_sb = singles.tile([128, HPC, C], F32R)        # wproj^T strips
            yt_sb = singles.tile([128, HPC, T], F32R)        # normalized y^T
            ident = singles.tile([128, 128], F32)
            ones = singles.tile([128, 1], F32R)
            masks = singles.tile([128, 4, 512], F32R)

            # HAM warm-up: ~5us of junk matmuls (no DMA dependency) so the
            # PE clock-gate reaches 8/8 while input DMAs stream in; results
            # are never read.
            wu = singles.tile([128, 128], BF16)
            nc.vector.memset(wu[:], 0.5)
            ps_wu = psum.tile([128, 128], F32, tag="ps", name="ps_wu")
            for _ in range(80):
                nc.tensor.matmul(
                    ps_wu[:], wu[:], wu[:], start=True, stop=True
                )

            nc.sync.dma_start(out=ident[:], in_=ident_d[:])
            nc.sync.dma_start(out=ones[:], in_=ones_d[:])
            nc.sync.dma_start(
                out=masks[:], in_=masks_d.rearrange("p (r f) -> p r f", f=512)
            )
            for cs in range(NCS):
                nc.sync.dma_start(
                    out=wq_sb[cs][:],
                    in_=wq[128 * cs : 128 * (cs + 1), :],
                )
            for rep in range(nrep):
              for b in range(B):
                # ---- load x^T strips for this batch element ----
                for cs in range(NCS):
                    nc.sync.dma_start(
                        out=xt_sb[cs][:],
                        in_=xt[128 * cs : 128 * (cs + 1), T * b : T * (b + 1)],
                    )

                # ---- QKV projection: fb 0..3 -> qT/kT, fb 4,5 -> v ----
                for fb in range(6):
                    ps_q = [
                        psum.tile([128, 512], F32, tag="ps", name="ps_q") for _ in range(4)
                    ]
                    for cs in range(NCS):
                        for tcn in range(4):
                            nc.tensor.matmul(
                                ps_q[tcn][:],
                                wq_sb[cs][:, 128 * fb : 128 * (fb + 1)],
                                xt_sb[cs][:, 512 * tcn : 512 * (tcn + 1)],
                                start=(cs == 0),
                                stop=(cs == NCS - 1),
                            )
                    if fb < 4:
                        for tcn in range(4):
                            nc.scalar.copy(
                                qkvt_sb[:, fb, 512 * tcn : 512 * (tcn + 1)],
                                ps_q[tcn][:],
                            )
                    else:
                        h = fb - 4
                        for tcn in range(4):
                            vt_t = vt_pool.tile([128, 512], F32)
                            nc.scalar.copy(vt_t[:], ps_q[tcn][:])
                            for s in range(4):
                                j = 4 * tcn + s
                                ps_tr = psum.tile([128, 128], F32, tag="ps", name="ps_tr")
                                nc.tensor.transpose(
                                    ps_tr[:],
                                    vt_t[:, 128 * s : 128 * (s + 1)],
                                    ident[:],
                                )
                                nc.vector.tensor_copy(
                                    vnat_sb[:, j, 128 * h : 128 * (h + 1)],
                                    ps_tr[:],
                                )

                if b == 0:
                    # wproj^T only needed at out-proj; keep it off the
                    # startup DMA critical path
                    nc.sync.dma_start(
                        out=wp_sb[:],
                        in_=wp.rearrange("(h p) o -> p h o", p=128),
                    )

                # ---- attention per head ----
                for h in range(HPC):
                    for cch in range(NCH):
                        nj = 4 * cch + 4  # causal: tk blocks 0..nj-1
                        q_sl = qkvt_sb[:, h, 512 * cch : 512 * (cch + 1)]
                        ps_sum = psum.tile([1, 512], F32, tag="ps", name="ps_sum")
                        ps_y = psum.tile([128, 512], F32, tag="ps", name="ps_y")

                        def scores(j, h=h, cch=cch, q_sl=q_sl):
                            # diagonal blocks with offset r>=2 are masked
                            # below column 128*r: compute only [lo:512)
                            # (N=256 keeps fp32r at full rate)
                            r = j - 4 * cch
                            lo = 256 if r >= 2 else 0
                            ps_s = psum.tile([128, 512], F32, tag="ps", name="ps_s")
                            nc.tensor.matmul(
                                ps_s[:, lo:512],
                                qkvt_sb[
                                    :, HPC + h, 128 * j : 128 * (j + 1)
                                ],
                                q_sl[:, lo:512],
                                start=True,
                                stop=True,
                            )
                            pt = pt_pool.tile([128, 512], F32R, tag="pt", name="pt")
                            nc.scalar.activation(
                                out=pt[:, lo:512],
                                in_=ps_s[:, lo:512],
                                func=EXP,
                                scale=SCALE,
                            )
                            if r >= 0:  # diagonal block: zero where tk > tq
                                nc.vector.tensor_mul(
                                    pt[:, lo:512],
                                    pt[:, lo:512],
                                    masks[:, r, lo:512],
                                )
                            return (pt, lo)

                        pipe = [scores(jj) for jj in range(min(3, nj))]
                        for j in range(nj):
                            pt_cur, lo = pipe.pop(0)
                            if j + 3 < nj:
                                pipe.append(scores(j + 3))
                            nc.tensor.matmul(
                                ps_sum[:, lo:512],
                                ones[:],
                                pt_cur[:, lo:512],
                                start=(j == 0),
                                stop=(j == nj - 1),
                                skip_group_check=True,
                            )
                            nc.tensor.matmul(
                                ps_y[:, lo:512],
                                vnat_sb[:, j, 128 * h : 128 * (h + 1)],
                                pt_cur[:, lo:512],
                                start=(j == 0),
                                stop=(j == nj - 1),
                                skip_group_check=True,
                            )

                        # decouple psum from the normalization chain:
                        # evict y and sums to SBUF fast, then normalize
                        yraw = yraw_pool.tile([128, 512], F32, name="yraw")
                        nc.vector.tensor_copy(yraw[:], ps_y[:])
                        recip = rc_pool.tile([1, 512], F32)
                        nc.vector.reciprocal(recip[:], ps_sum[:])
                        rb = dram_pool.tile([1, 512], F32, name="rb")
                        nc.sync.dma_start(out=rb[:], in_=recip[:])
                        bc = bc_pool.tile([128, 512], F32)
                        nc.sync.dma_start(
                            out=bc[:], in_=rb[:].to_broadcast([128, 512])
                        )
                        nc.vector.tensor_mul(
                            yt_sb[:, h, 512 * cch : 512 * (cch + 1)],
                            yraw[:],
                            bc[:],
                        )

                # ---- output projection (partial over this core's 256 f) ----
                for tb in range(NB):
                    for half in range(2):
                        out_t = out_pool.tile(
                            [128, 1024], F16, tag="outs", name="out_t"
                        )
                        for k in range(2):
                            oc = 2 * half + k
                            ps_o = psum.tile(
                                [128, 512], F32, tag="ps", name="ps_o"
                            )
                            for hh in range(HPC):
                                nc.tensor.matmul(
                                    ps_o[:],
                                    yt_sb[:, hh, 128 * tb : 128 * (tb + 1)],
                                    wp_sb[:, hh, 512 * oc : 512 * (oc + 1)],
                                    start=(hh == 0),
                                    stop=(hh == HPC - 1),
                                )
                            # alternate eviction engine: ACT and DVE
                            dst = out_t[:, 512 * k : 512 * (k + 1)]
                            if oc % 2 == 0:
                                nc.scalar.copy(dst, ps_o[:])
                            else:
                                nc.vector.tensor_copy(dst, ps_o[:])
                        nc.sync.dma_start(
                            out=outp[
                                T * b + 128 * tb : T * b + 128 * (tb + 1),
                                1024 * half : 1024 * (half + 1),
                            ],
                            in_=out_t[:],
                        )

    nc.compile()
    return nc


def get_nc(nrep=1):
    key = f"nc{nrep}"
    if key not in _CACHE:
        _CACHE[key] = _build_nc(nrep)
    return _CACHE[key]


def make_in_maps(x, w_attn, w_proj):
    """Host-side sharding: transpose + bf16 cast + per-core weight slices."""
    xt = np.ascontiguousarray(
        x.reshape(BT, C).T.astype(ml_dtypes.bfloat16)
    )  # [C, BT]
    p = np.arange(128)[:, None]
    f = np.arange(512)[None, :]
    masks = np.concatenate(
        [(p + 128 * r <= f).astype(np.float32) for r in range(4)], axis=1
    )  # [128, 2048]
    ident = np.eye(128, dtype=np.float32)
    ones = np.ones((128, 1), dtype=np.float32)
    in_maps = []
    for core in range(NCORES):
        h0 = HPC * core
        rows = np.concatenate(
            [
                w_attn[HD * h0 : HD * (h0 + HPC), :],          # q heads
                w_attn[C + HD * h0 : C + HD * (h0 + HPC), :],  # k heads
                w_attn[2 * C + HD * h0 : 2 * C + HD * (h0 + HPC), :],  # v
            ],
            axis=0,
        )  # [768, C]
        wq_c = np.ascontiguousarray(rows.T.astype(ml_dtypes.bfloat16))
        wp_c = np.ascontiguousarray(
            w_proj[:, FQ * core : FQ * (core + 1)].T.astype(np.float32)
        )  # [256, C]
        in_maps.append(
            {
                "xt": xt,
                "wq": wq_c,
                "wp": wp_c,
                "masks": masks,
                "ident": ident,
                "ones": ones,
            }
        )
    return in_maps


def kernel(x, w_attn, w_proj):
    import os
    from concourse.bass_utils import run_bass_kernel_spmd

    x = np.asarray(x, dtype=np.float32)
    w_attn = np.asarray(w_attn, dtype=np.float32)
    w_proj = np.asarray(w_proj, dtype=np.float32)

    nc = get_nc()
    in_maps = make_in_maps(x, w_attn, w_proj)
    try:
        res = run_bass_kernel_spmd(nc, in_maps, core_ids=list(range(NCORES)))
    except ModuleNotFoundError:
        # BASS_TRACE set but the axon NTFF profiling hook is unavailable
        # in this container; rerun without tracing.
        os.environ["BASS_NEVER_TRACE"] = "1"
        res = run_bass_kernel_spmd(nc, in_maps, core_ids=list(range(NCORES)))
    acc = np.zeros((BT, C), dtype=np.float32)
    for r in res.results:
        acc += r["outp"].astype(np.float32)
    return acc.reshape(B, T, C)


if __name__ == "__main__":
    nc = get_nc()
    print("built + compiled OK")
